# revision 10
# baseline (speedup 1.0000x reference)
"""Trainium2 Bass kernel for PVT-style spatial-reduction attention with LoRA.

Sharding: 8 cores = (batch b in {0,1}) x (head-pair pp in {0..3}). Each core
is fully independent (NO collectives): it computes the full spatial-reduction
conv + LayerNorm for its batch (replicated across the 4 cores of the batch
group), its head-pair's q/k/v, attention for its 2 heads, and a partial
output projection [C, N]. The host sums the 4 partial projections per batch.

The whole per-rep body sits inside a tc.For_i hardware loop, so the NEFF
contains ONE copy of the instruction stream regardless of reps and the
per-rep marginal cost is pure device execution time.

Scheduling notes (engine queues are FIFO per engine):
- conv accumulates ci-outer so compute starts after the first 1MB x DMA.
- q-projection is emitted between the LN stats matmuls and the LN scalar
  chain so the tensor engine never waits on the (serial) LN math.
- the softmax divide + output projection for query-block qb-1 are emitted
  before the attention chain of qb (one-stage software pipeline), hiding
  the reciprocal latency.

Host folds: LoRA into the dense weights, softmax scale into Wq/bq, LN
gamma/beta into Wk/Wv and the output bias, k-bias dropped (softmax-
invariant), v-bias folded into the output bias. Softmax denominators come
from an all-ones column appended to the stationary V operand; max-
subtraction is skipped (logits are bounded ~|2|).
"""
import sys
for _p in ('/opt/trn_rl_repo', '/root/.axon_site/_ro/trn_rl_repo'):
    if _p not in sys.path:
        sys.path.insert(0, _p)

import numpy as np

B, N, C, HEAD, SR, R = 2, 4096, 512, 8, 2, 8
HH = WW = 64
DH = C // HEAD               # 64
M = (HH // SR) * (WW // SR)  # 1024 kv positions
LN_EPS = 1e-5
NCORES = 8

_cached = {}


def _build_nc(reps=1, phases='all'):
    from concourse import bacc, tile, mybir
    import concourse.bass as bass_mod

    f32 = mybir.dt.float32
    f16 = mybir.dt.float16
    f8 = mybir.dt.float8e4
    DR = mybir.MatmulPerfMode.DoubleRow
    ACT = mybir.ActivationFunctionType

    nc = bacc.Bacc("TRN2", target_bir_lowering=False, debug=False,
                   num_devices=NCORES)
    xT_d = nc.dram_tensor("xT", [C, N], f16, kind="ExternalInput")
    wsr_d = nc.dram_tensor("wsr", [16, 128, C], f16, kind="ExternalInput")
    wqkv_d = nc.dram_tensor("wqkv", [4, 128, 384], f16, kind="ExternalInput")
    wp_d = nc.dram_tensor("wp", [128, C], f16, kind="ExternalInput")
    bias_d = nc.dram_tensor("bias", [128, 6], f32, kind="ExternalInput")
    out_d = nc.dram_tensor("outT", [C, N], f16, kind="ExternalOutput")

    with tile.TileContext(nc) as tc:
        with tc.tile_pool(name="w", bufs=1) as wpool:
            # ---- weights: loaded once, reused across reps ----
            wsr = wpool.tile([128, 16, C], f16)
            nc.sync.dma_start(wsr[:], wsr_d.rearrange("g p n -> p g n"))
            wqkv = wpool.tile([128, 4, 384], f16)
            nc.sync.dma_start(wqkv[:], wqkv_d.rearrange("t p n -> p t n"))
            wp = wpool.tile([128, C], f16)
            nc.sync.dma_start(wp[:], wp_d[:])
            bias = wpool.tile([128, 6], f32)
            nc.sync.dma_start(bias[:], bias_d[:])
            ones_invC = wpool.tile([128, 1], f16)
            nc.vector.memset(ones_invC[:], 1.0 / C)
            ones_row = wpool.tile([1, 128], f16)
            nc.vector.memset(ones_row[:], 1.0)
            ones33 = wpool.tile([33, 64], f16)
            nc.vector.memset(ones33[:], 1.0)
            bq = bias[:, 0:1]
            eps = bias[0:1, 5:6]

            with tc.tile_pool(name="m", bufs=1) as mp, \
                 tc.tile_pool(name="pex", bufs=3) as pexp:
                with tc.For_i(0, reps):
                    # ---- x load: 4 chunks so conv can start early ----
                    xt = [mp.tile([128, N], f16, tag=f"x{ct}",
                                  name=f"x{ct}")
                          for ct in range(4)]
                    for ct in range(4):
                        nc.sync.dma_start(
                            xt[ct][:], xT_d[ct * 128:(ct + 1) * 128, :])
                    xv = [xt[ct].rearrange("p (ph a pw b) -> p ph a pw b",
                                           ph=32, a=2, pw=32, b=2)
                          for ct in range(4)]

                    # ---- conv: full z [128, 4ct, M] for this batch ----
                    z = mp.tile([128, 4, M], f16, tag="z")
                    zsq = mp.tile([128, 4, M], f16, tag="zsq")
                    with tc.tile_pool(name="psc", bufs=2,
                                      space="PSUM") as pse:
                        for co in range(4):
                            accs = [pse.tile([128, 512], f32, tag=f"cv{qc}",
                                             name=f"cv{qc}")
                                    for qc in range(2)]
                            step = 0
                            for ci in range(4):      # ci-outer: early start
                                for dydx in range(4):
                                    g = dydx * 4 + ci
                                    dy, dx = dydx // 2, dydx % 2
                                    for qc in range(2):
                                        rhs = xv[ci][:,
                                                     qc * 16:(qc + 1) * 16,
                                                     dy, :, dx]
                                        nc.tensor.matmul(
                                            accs[qc][:],
                                            wsr[:, g,
                                                co * 128:(co + 1) * 128],
                                            rhs, start=(step == 0),
                                            stop=(step == 15))
                                    step += 1
                            for qc in range(2):
                                nc.scalar.activation(
                                    out=z[:, co, qc * 512:(qc + 1) * 512],
                                    in_=accs[qc][:], func=ACT.Identity,
                                    bias=bias[:, 1 + co:2 + co], scale=1.0)
                            nc.vector.tensor_mul(zsq[:, co, :], z[:, co, :],
                                                 z[:, co, :])

                    # ---- LN stats matmuls, then q-proj (overlaps LN math),
                    #      then LN chain + broadcast + normalize ----
                    mean = mp.tile([1, M], f32, tag="mean")
                    e2 = mp.tile([1, M], f32, tag="e2")
                    rs16 = mp.tile([1, M], f16, tag="rs16")
                    sh16 = mp.tile([1, M], f16, tag="sh16")
                    qT = mp.tile([128, N], f16, tag="qT")
                    kT = mp.tile([128, M], f16, tag="kT")
                    v65 = mp.tile([128, 16, 80], f16, tag="v65")
                    with tc.tile_pool(name="psb", bufs=2,
                                      space="PSUM") as pse:
                        for half in range(2):
                            sl = slice(half * 512, (half + 1) * 512)
                            pm = pse.tile([1, 512], f32, tag="st", name="pm")
                            for ct in range(4):
                                nc.tensor.matmul(pm[:], ones_invC[:],
                                                 z[:, ct, sl],
                                                 start=(ct == 0),
                                                 stop=(ct == 3))
                            nc.vector.tensor_copy(mean[:, sl], pm[:])
                            pq = pse.tile([1, 512], f32, tag="st", name="pq")
                            for ct in range(4):
                                nc.tensor.matmul(pq[:], ones_invC[:],
                                                 zsq[:, ct, sl],
                                                 start=(ct == 0),
                                                 stop=(ct == 3))
                            nc.vector.tensor_copy(e2[:, sl], pq[:])

                        # q-projection: independent of LN — keeps tensor busy
                        for qb in range(8):
                            sl = slice(qb * 512, (qb + 1) * 512)
                            ps = pse.tile([128, 512], f32, tag="mm",
                                          name="psq")
                            for ct in range(4):
                                nc.tensor.matmul(ps[:], wqkv[:, ct, 0:128],
                                                 xt[ct][:, sl],
                                                 start=(ct == 0),
                                                 stop=(ct == 3))
                            nc.scalar.activation(out=qT[:, sl], in_=ps[:],
                                                 func=ACT.Identity, bias=bq,
                                                 scale=1.0)

                        # LN scalar chain (runs on DVE/ACT under q-proj)
                        m2 = mp.tile([1, M], f32, tag="m2")
                        nc.vector.tensor_mul(m2[:], mean[:], mean[:])
                        nc.vector.tensor_sub(e2[:], e2[:], m2[:])   # var
                        nc.scalar.activation(out=e2[:], in_=e2[:],
                                             func=ACT.Sqrt, bias=eps,
                                             scale=1.0)
                        nc.vector.reciprocal(e2[:], e2[:])          # rstd
                        nc.vector.tensor_mul(mean[:], mean[:], e2[:])
                        nc.scalar.mul(mean[:], mean[:], -1.0)       # -mu*rstd
                        nc.vector.tensor_copy(rs16[:], e2[:])
                        nc.vector.tensor_copy(sh16[:], mean[:])

                        # broadcast LN scale/shift + normalize z in place
                        for half in range(2):
                            sl = slice(half * 512, (half + 1) * 512)
                            bcs = pse.tile([128, 512], f32, tag="bc",
                                           name="bcs")
                            nc.tensor.matmul(bcs[:], ones_row[:],
                                             rs16[:, sl],
                                             start=True, stop=True)
                            bct = pse.tile([128, 512], f32, tag="bc",
                                           name="bct")
                            nc.tensor.matmul(bct[:], ones_row[:],
                                             sh16[:, sl],
                                             start=True, stop=True)
                            for ct in range(4):
                                nc.vector.tensor_mul(z[:, ct, sl],
                                                     z[:, ct, sl], bcs[:])
                                nc.vector.tensor_add(z[:, ct, sl],
                                                     z[:, ct, sl], bct[:])

                        # ---- k / v projections ----
                        for kb in range(2):
                            sl = slice(kb * 512, (kb + 1) * 512)
                            ps = pse.tile([128, 512], f32, tag="mm",
                                          name="psk")
                            for ct in range(4):
                                nc.tensor.matmul(ps[:], wqkv[:, ct, 128:256],
                                                 z[:, ct, sl],
                                                 start=(ct == 0),
                                                 stop=(ct == 3))
                            nc.vector.tensor_copy(kT[:, sl], ps[:])
                        nc.vector.memset(v65[:], 1.0)
                        for kt in range(8):
                            ps = pse.tile([128, 128], f32, tag="vv",
                                          name="psv")
                            for ct in range(4):
                                nc.tensor.matmul(
                                    ps[:], z[:, ct, kt * 128:(kt + 1) * 128],
                                    wqkv[:, ct, 256:384],
                                    start=(ct == 0), stop=(ct == 3))
                            vd = v65[:, 0, :]
                            vdst = bass_mod.AP(
                                tensor=vd.tensor,
                                offset=vd.offset + kt * 80,
                                ap=[list(vd.ap[0]), [8 * 80, 2], [1, 64]])
                            nc.vector.tensor_copy(
                                vdst, ps.rearrange("p (h d) -> p h d", h=2))

                    # ---- attention + pipelined divide/projection ----
                    ob = mp.tile([128, 4, N], f16, tag="ob")
                    oview = out_d.rearrange("(t p) n -> p t n", p=128)
                    with tc.tile_pool(name="psa", bufs=2,
                                      space="PSUM") as pse:
                        prev = None

                        def emit_divide(att_p, rc2_p, qb_p):
                            # scale both heads by 1/denominator (broadcast
                            # via outer-product matmul)
                            for h in range(2):
                                hs = slice(64 * h, 64 * h + 64)
                                hp = 32 * h
                                psb = pse.tile([64, 512], f32, tag="b",
                                               name="psb")
                                nc.tensor.matmul(psb[:],
                                                 ones33[hp:hp + 1, :],
                                                 rc2_p[hp:hp + 1, :],
                                                 start=True, stop=True)
                                nc.vector.tensor_mul(att_p[hs, :],
                                                     att_p[hs, :], psb[:])

                        def emit_proj(att_p, rc2_p, qb_p):
                            qsl = slice(qb_p * 512, (qb_p + 1) * 512)
                            for cb in range(4):
                                pp = pse.tile([128, 512], f32, tag="mm",
                                              name="pp")
                                nc.tensor.matmul(
                                    pp[:], wp[:, cb * 128:(cb + 1) * 128],
                                    att_p[:], start=True, stop=True)
                                nc.vector.tensor_copy(ob[:, cb, qsl], pp[:])
                                nc.sync.dma_start(oview[:, cb, qsl],
                                                  ob[:, cb, qsl])

                        def emit_chain(att, den2, h, qb):
                            qsl = slice(qb * 512, (qb + 1) * 512)
                            hs = slice(64 * h, 64 * h + 64)
                            pso = pse.tile([65, 512], f32, tag="o",
                                           name="pso")
                            for kt in range(8):
                                psl = pse.tile([128, 512], f32, tag="l",
                                               name="psl")
                                nc.tensor.matmul(
                                    psl[:],
                                    kT[hs, kt * 128:(kt + 1) * 128],
                                    qT[hs, qsl], start=True, stop=True)
                                pex = pexp.tile([128, 512], f16,
                                                tag="pex")
                                nc.scalar.activation(out=pex[:],
                                                     in_=psl[:],
                                                     func=ACT.Exp)
                                nc.tensor.matmul(
                                    pso[:],
                                    v65[:, 8 * h + kt, 0:65],
                                    pex[:], start=(kt == 0), stop=(kt == 7))
                            nc.scalar.copy(att[hs, :], pso[0:64, :])
                            nc.scalar.copy(den2[32 * h:32 * h + 1, :],
                                           pso[64:65, :])

                        for qb in range(8):
                            att = pexp.tile([128, 512], f16, tag="att")
                            den2 = pexp.tile([33, 512], f32, tag="den")
                            if prev is not None:
                                emit_divide(*prev)
                            emit_chain(att, den2, 0, qb)
                            if prev is not None:
                                emit_proj(*prev)
                            emit_chain(att, den2, 1, qb)
                            rc2 = pexp.tile([33, 512], f16, tag="rc")
                            with nc.allow_low_precision(reason="denom f16"):
                                nc.vector.reciprocal(rc2[:], den2[:])
                            prev = (att, rc2, qb)
                        emit_divide(*prev)
                        emit_proj(*prev)

    nc.compile()
    return nc


def _host_prep(inputs):
    x = inputs["x"]; Wq = inputs["Wq"]; bq = inputs["bq"]
    Wkv = inputs["Wkv"]; bkv = inputs["bkv"]
    Wproj = inputs["Wproj"]; bproj = inputs["bproj"]
    Aq = inputs["Aq"]; Bq = inputs["Bq"]; Av = inputs["Av"]; Bv = inputs["Bv"]
    Wsr = inputs["Wsr"]; bsr = inputs["bsr"]
    gamma = inputs["gamma"]; beta = inputs["beta"]
    scale = DH ** -0.5

    Wq_eff = ((Wq + Aq @ Bq) * scale).astype(np.float32)
    bq_eff = (bq * scale).astype(np.float32)
    Wk = Wkv[:, :C]; Wv = Wkv[:, C:]
    AvBv = (Av @ Bv).astype(np.float32)
    Wk_g = (gamma[:, None] * (Wk + AvBv)).astype(np.float32)
    Wv_g = (gamma[:, None] * (Wv + AvBv)).astype(np.float32)
    bv_eff = (beta @ (Wv + AvBv) + bkv[C:]).astype(np.float32)
    bfinal = (bproj + bv_eff @ Wproj).astype(np.float32)
    Wsr_flat = np.ascontiguousarray(Wsr.reshape(4 * C, C), np.float32)

    in_maps = []
    for core in range(NCORES):
        b, p = core // 4, core % 4
        cols = slice(128 * p, 128 * p + 128)
        wqkv = np.concatenate([Wq_eff[:, cols], Wk_g[:, cols], Wv_g[:, cols]],
                              axis=1)  # [512, 384]
        bias = np.zeros((128, 6), np.float32)
        bias[:, 0] = bq_eff[cols]
        for co in range(4):
            bias[:, 1 + co] = bsr[co * 128:(co + 1) * 128]
        bias[:, 5] = LN_EPS
        m = {
            "xT": np.ascontiguousarray(x[b].T),                  # [512, N]
            "wsr": Wsr_flat.reshape(16, 128, C),
            "wqkv": np.ascontiguousarray(wqkv).reshape(4, 128, 384),
            "wp": np.ascontiguousarray(Wproj[cols, :]),
            "bias": bias,
        }
        f16keys = {"xT", "wsr", "wqkv", "wp"}
        in_maps.append({k: np.ascontiguousarray(
            v, np.float16 if k in f16keys else np.float32)
            for k, v in m.items()})
    return in_maps, bfinal


def run_device(inputs, reps=1, phases='all'):
    from concourse.bass_utils import run_bass_kernel_spmd
    key = f"nc{reps}{phases}"
    if key not in _cached:
        _cached[key] = _build_nc(reps, phases)
    nc = _cached[key]
    in_maps, bfinal = _host_prep(inputs)
    res = run_bass_kernel_spmd(nc, in_maps, core_ids=list(range(NCORES)))
    return res, bfinal


def kernel(**inputs):
    inputs = {k: np.asarray(v) for k, v in inputs.items()}
    res, bfinal = run_device(inputs, reps=1)
    out = np.zeros((B, N, C), np.float32)
    for b in range(B):
        acc = np.zeros((C, N), np.float32)
        for p in range(4):
            acc += np.asarray(res.results[4 * b + p]["outT"], np.float32)
        out[b] = acc.T + bfinal[None, :]
    return out
